# revision 60
# baseline (speedup 1.0000x reference)
"""Trainium2 kernel for nn_Graph_41609643163904.

The reference op is a sequential per-cell scatter sweep over a 48x48 grid:
for x in 2..45, y in 2..45 (x outer): read center v, zero it, add v*W[y,x]
to the 5x5 neighborhood.  Every step is linear in the grid, so the whole
sweep is one fixed linear operator M (2304x2304) depending only on the
weights.  We build M on the host, then the device work is a batched matmul
out = in @ M^T, data-parallel over the 8192-sample batch across 8
NeuronCores (1024 samples/core), zero comm.

Device-side structure (v4):
  * x-major re-flattening exposes the sweep's causal cone as block
    sparsity: per 128-wide j-tile only a prefix of k-tiles is nonzero
    (188 of 324 blocks).
  * two-tier precision, tuned on the host against an exact product:
      - slow tier: bf16 weights x bf16 activations, fp32 PSUM.
      - fast tier (adjacent k-tile pairs): e4m3 DoubleRow matmuls
        contract 256 rows per pass (~2x PE throughput).  Pairs are
        picked greedily by MEASURED output-error increase until the
        simulated relative error reaches ERR_TARGET (limit is 2e-2).
    The fast tier accumulates in its own PSUM bank (weights pre-scaled
    per-tile for e4m3 range, x pre-scaled by 32); the merge
    out = psA + psB/(32*SC_t) rides on the PSUM->SBUF drain.
  * out^T orientation: M' blocks stationary, batch (512-wide moving
    operand) streams through; m-outer so each tile's first half drains
    while the second half computes.
  * Few, large DMAs: M packed per 3-tile group in consumption order
    (sync ring), x resident in two k-major mega-tiles loaded in chunks
    (bf16 on scalar, fp8 on sync), one store per tile (gpsimd early,
    scalar late).  Small tiles are processed last to shorten the tail.
"""

import os

import numpy as np
import ml_dtypes

SIZE = 48
D = 2
K = 5
N = SIZE * SIZE          # 2304
B = 8192
NCORES = 8
BS = B // NCORES         # 1024 samples per core

P = 128
NK = N // P              # 18 k-tiles
NJ = N // P              # 18 j-tiles
MW = 512                 # moving-operand width (max for fp32 PSUM bank)
NM = BS // MW            # 2 m-tiles per core

ERR_TARGET = 1.90e-2     # budget for quantization error (limit is 2e-2)
SIM_NS = 512             # samples used in the host error simulation
X4SC = 32.0              # x fp8 pre-scale (keeps small x out of subnormals)

# Processing order: small tiles 2 and 3 last so the final drain is short.
ORDER = [0, 1] + list(range(4, NJ)) + [3, 2]
# Position ranges forming one M-stream DMA group each (first two small so
# the PE starts fast).
GROUPS = [(0, 1), (1, 3), (3, 6), (6, 9), (9, 12), (12, 15), (15, 18)]
# xb chunk boundaries in k (first small for a fast start).
XB_CHUNKS = [(0, 1), (1, 4), (4, 7), (7, 10), (10, 13), (13, 16), (16, 18)]
X4_CHUNKS = [(0, 2), (2, 6), (6, 10), (10, 14), (14, 18)]

# Structural nonzero k-tile prefix per 128-wide j-tile (x-major layout).
KPREF = tuple(
    min(NK, -(-(SIZE * ((P * (t + 1) - 1) // SIZE + 3)) // P)) for t in range(NJ)
)


def _build_M(weights: np.ndarray) -> np.ndarray:
    """Compose the 1936 per-cell updates into one (N, N) operator, fp64."""
    M = np.eye(N, dtype=np.float64)
    w = weights.astype(np.float64)
    for x in range(D, SIZE - D):
        for y in range(D, SIZE - D):
            c = y * SIZE + x
            wc = w[y, x]
            rc = M[c].copy()
            for dy in range(-D, D + 1):
                r0 = c + dy * SIZE - D
                wrow = wc[dy + D]
                if dy == 0:
                    M[r0:r0 + D] += np.outer(wrow[:D], rc)
                    M[r0 + D + 1:r0 + K] += np.outer(wrow[D + 1:], rc)
                else:
                    M[r0:r0 + K] += np.outer(wrow, rc)
            M[c] = wc[D, D] * rc
    return M


def _xmajor_idx():
    n = np.arange(N)
    return (n % SIZE) * SIZE + n // SIZE


def _q(a, dt):
    return a.astype(dt).astype(np.float32)


def _pick_fast_pairs(Mp: np.ndarray, xP: np.ndarray):
    """Greedy: convert adjacent k-tile pairs to e4m3 DoubleRow, picking the
    pair with the smallest measured output-error increase each round, until
    the simulated total relative error reaches ERR_TARGET.

    Returns (fast: dict t -> list of k0, SC: per-tile weight scale).
    """
    f32 = np.float32
    MT = Mp.T
    cand = []   # (t, k0)
    stats = {t: [] for t in range(NJ)}   # (energy, max_entry)
    early = set(ORDER[:6])   # keep early tiles bf16-only: their fp8 x
    # would land in the HBM-saturated startup window.
    for t in range(NJ):
        if t in early:
            continue
        for k0 in range(KPREF[t] - 1):
            blk = MT[k0 * P:(k0 + 2) * P, t * P:(t + 1) * P]
            stats[t].append((float((blk ** 2).sum()),
                             float(np.abs(blk).max())))
            cand.append((t, k0))
    # Per-tile e4m3 scale sized for the lower-energy half of that tile's
    # candidate pairs (the ones the greedy actually converts); big-entry
    # pairs that would clip get a large measured error and are skipped.
    SC = []
    for t in range(NJ):
        st = sorted(stats[t])
        if not st:
            SC.append(1.0)
            continue
        mx = max(m for _, m in st[:max(1, len(st) // 2)])
        SC.append(float(2.0 ** np.floor(np.log2(240.0 / mx))))

    xs = xP[:SIM_NS].astype(f32)
    xb = _q(xs, ml_dtypes.bfloat16)
    x4 = _q(np.clip(xs * X4SC, -240, 240), ml_dtypes.float8_e4m3) / X4SC
    Mf32 = Mp.astype(f32)
    out_exact = xs.astype(np.float64) @ Mp.T
    Mb = _q(Mf32, ml_dtypes.bfloat16)
    err = (xb @ Mb.T).astype(np.float64) - out_exact   # [ns, N]
    den = np.linalg.norm(out_exact)

    # Per-candidate error-delta vectors (confined to j-tile t's columns).
    dvec = {}
    for t, k0 in cand:
        js = slice(t * P, (t + 1) * P)
        ks = slice(k0 * P, (k0 + 2) * P)
        blk = Mf32[js, ks]
        blk_q = _q(np.clip(blk * SC[t], -240, 240),
                   ml_dtypes.float8_e4m3) / SC[t]
        dvec[(t, k0)] = (x4[:, ks] @ blk_q.T
                         - xb[:, ks] @ Mb[js, ks].T).astype(np.float64)

    err_sq = float(np.linalg.norm(err) ** 2)
    fast = {t: [] for t in range(NJ)}
    used = {t: set() for t in range(NJ)}
    alive = dict(dvec)
    while alive:
        best, best_inc, best_d = None, None, None
        for (t, k0), d in alive.items():
            inc = float(2.0 * np.tensordot(err[:, t * P:(t + 1) * P], d)
                        + np.linalg.norm(d) ** 2)
            if best_inc is None or inc < best_inc:
                best, best_inc, best_d = (t, k0), inc, d
        t, k0 = best
        if np.sqrt(max(err_sq + best_inc, 0.0)) / den > ERR_TARGET:
            break
        err_sq += best_inc
        err[:, t * P:(t + 1) * P] += best_d
        fast[t].append(k0)
        used[t].update((k0, k0 + 1))
        alive = {(tt, kk): d for (tt, kk), d in alive.items()
                 if not (tt == t and (kk in used[t] or kk + 1 in used[t]))}
    return fast, SC


def _build_device_kernel(slow_ks, fast_k0, ns_off, nq_off, SC,
                         ntot_s, ntot_q):
    import concourse.mybir as mybir
    from concourse import bacc
    from concourse.tile import TileContext

    f32 = mybir.dt.float32
    bf16 = mybir.dt.bfloat16
    f8e4 = mybir.dt.float8e4
    Copy = mybir.ActivationFunctionType.Copy
    DR = mybir.MatmulPerfMode.DoubleRow

    nc = bacc.Bacc()
    xb = nc.dram_tensor("xb", [N, BS], bf16, kind="ExternalInput")
    x4 = nc.dram_tensor("x4", [N, BS], f8e4, kind="ExternalInput")
    ms = nc.dram_tensor("ms", [P, max(ntot_s, 1) * P], bf16,
                        kind="ExternalInput")
    mf = nc.dram_tensor("mf", [P, max(ntot_q, 1) * 2 * P], f8e4,
                        kind="ExternalInput")
    outT = nc.dram_tensor("outT", [N, BS], bf16, kind="ExternalOutput")

    xb_r = xb.rearrange("(k p) m -> p k m", p=P)
    x4_r = x4.rearrange("(k p) m -> p k m", p=P)

    # Per-group block ranges (in packed-block units).
    g_ns = []   # (start_block, n_blocks)
    g_nq = []
    for (p0, p1) in GROUPS:
        ts = ORDER[p0:p1]
        g_ns.append((ns_off[ts[0]], sum(len(slow_ks[t]) for t in ts)))
        g_nq.append((nq_off[ts[0]], sum(len(fast_k0[t]) for t in ts)))

    with TileContext(nc) as tc:
        with (
            tc.tile_pool(name="xpool", bufs=1) as xpool,
            tc.tile_pool(name="mpool", bufs=1) as mpool,
            tc.tile_pool(name="fpool", bufs=1) as fpool,
            tc.tile_pool(name="opool", bufs=3) as opool,
            tc.tile_pool(name="tpool", bufs=4) as tpool,
            tc.tile_pool(name="pspool", bufs=2, space="PSUM") as pspool,
        ):
            xball = xpool.tile([P, NK, BS], bf16, tag="xball", name="xball")
            x4all = xpool.tile([P, NK, BS], f8e4, tag="x4all", name="x4all")

            # Issue ALL load DMAs up front, in consumption order per ring.
            # No engine-stream blocking: rings deliver FIFO at full rate
            # while the PE consumes behind them.  (Merges on the scalar
            # engine then sit after all its load issues.)
            for c0, c1 in XB_CHUNKS:
                nc.scalar.dma_start(out=xball[:, c0:c1, :],
                                    in_=xb_r[:, c0:c1, :])
            ms_g = {}
            mf_g = {}

            def issue_group(g):
                s0, sn = g_ns[g]
                q0, qn = g_nq[g]
                if sn:
                    mt = mpool.tile([P, sn * P], bf16, tag=f"ms{g}",
                                    name=f"msg{g}")
                    nc.sync.dma_start(out=mt[:],
                                      in_=ms[:, s0 * P:(s0 + sn) * P])
                    ms_g[g] = (mt, s0)
                if qn:
                    ft = fpool.tile([P, qn * 2 * P], f8e4, tag=f"mf{g}",
                                    name=f"mfg{g}")
                    nc.sync.dma_start(
                        out=ft[:],
                        in_=mf[:, q0 * 2 * P:(q0 + qn) * 2 * P])
                    mf_g[g] = (ft, q0)

            sync_seq = [("g", 0), ("g", 1), ("g", 2), ("g", 3), ("x4", 0),
                        ("x4", 1), ("x4", 2), ("g", 4), ("x4", 3), ("g", 5),
                        ("x4", 4), ("g", 6)]
            for kind, i in sync_seq:
                if kind == "g":
                    issue_group(i)
                else:
                    c0, c1 = X4_CHUNKS[i]
                    nc.sync.dma_start(out=x4all[:, c0:c1, :],
                                      in_=x4_r[:, c0:c1, :])

            pos2g = {}
            for g, (p0, p1) in enumerate(GROUPS):
                for p in range(p0, p1):
                    pos2g[p] = g

            for pos, t in enumerate(ORDER):
                g = pos2g[pos]

                sks = slow_ks[t]
                fks = fast_k0[t]
                ns_t, nq_t = len(sks), len(fks)

                ot = opool.tile([P, BS], bf16, tag="o", name=f"o{t}")
                psA = psB = None
                if ns_t:
                    psA = {m: pspool.tile([P, MW], f32, tag=f"psA{m}",
                                          name=f"psA{t}_{m}")
                           for m in range(NM)}
                    mst, s0 = ms_g[g]
                    soff = ns_off[t] - s0
                if nq_t:
                    psB = {m: pspool.tile([P, MW], f32, tag=f"psB{m}",
                                          name=f"psB{t}_{m}")
                           for m in range(NM)}
                    mft, q0 = mf_g[g]
                    qoff = nq_off[t] - q0

                # m-outer: m=0's accumulation stops (and its merge) while
                # m=1's matmuls still stream.
                for m in range(NM):
                    for i, k in enumerate(sks):
                        nc.tensor.matmul(
                            psA[m][:],
                            lhsT=mst[:, (soff + i) * P:(soff + i + 1) * P],
                            rhs=xball[:, k, m * MW:(m + 1) * MW],
                            start=(i == 0),
                            stop=(i == ns_t - 1),
                        )
                    for q, k0 in enumerate(fks):
                        qq = qoff + q
                        nc.tensor.matmul(
                            psB[m][:],
                            lhsT=mft[:, qq * 2 * P:(qq + 1) * 2 * P].rearrange(
                                "p (two j) -> p two j", two=2),
                            rhs=x4all[:, k0:k0 + 2, m * MW:(m + 1) * MW],
                            start=(q == 0),
                            stop=(q == nq_t - 1),
                            perf_mode=DR,
                        )
                    osl = ot[:, m * MW:(m + 1) * MW]
                    fscale = 1.0 / (X4SC * SC[t])
                    # Merges live on DVE only: putting them on the scalar
                    # engine blocks its DMA issues behind PE waits
                    # (priority inversion that starves the x stream).
                    if ns_t and nq_t:
                        tmp = tpool.tile([P, MW], bf16, tag="tmp",
                                         name=f"tmp{t}_{m}")
                        nc.vector.tensor_scalar_mul(tmp[:], psB[m][:],
                                                    fscale)
                        nc.vector.tensor_add(osl, psA[m][:], tmp[:])
                    elif ns_t:
                        nc.vector.tensor_copy(osl, psA[m][:])
                    else:
                        nc.vector.tensor_scalar_mul(osl, psB[m][:], fscale)
                # one store per tile; slow SWDGE early, scalar ring late.
                st_eng = nc.gpsimd if pos < 9 else nc.scalar
                st_eng.dma_start(out=outT[t * P:(t + 1) * P, :], in_=ot[:])
    if not nc.is_finalized():
        nc.finalize()
    return nc


def kernel(inputs: np.ndarray, weights: np.ndarray) -> np.ndarray:
    from concourse.bass_utils import run_bass_kernel_spmd

    inputs = np.ascontiguousarray(inputs, dtype=np.float32)
    weights = np.ascontiguousarray(weights, dtype=np.float32)

    M = _build_M(weights)
    idx = _xmajor_idx()
    Mp = M[np.ix_(idx, idx)]
    xP = inputs.reshape(B, SIZE, SIZE).transpose(0, 2, 1).reshape(B, N)

    fast, SC = _pick_fast_pairs(Mp, xP)
    if os.environ.get("KERNEL_TRACE"):
        print(f"fast pairs: {sum(len(v) for v in fast.values())} "
              f"SC: {sorted(set(SC))}")

    slow_ks, fast_k0 = [], []
    for t in range(NJ):
        fks = sorted(fast[t])
        in_fast = {k for k0 in fks for k in (k0, k0 + 1)}
        slow_ks.append([k for k in range(KPREF[t]) if k not in in_fast])
        fast_k0.append(fks)

    # Host packing in PROCESSING order (ORDER), so each 3-tile group is one
    # contiguous DMA.  ms: bf16 slow blocks.  mf: e4m3 fast pairs, scaled
    # per tile.
    MTf = np.ascontiguousarray(Mp.T.astype(np.float32))
    ms_cols, mf_cols = [], []
    ns_off = [0] * NJ
    nq_off = [0] * NJ
    ns_tot = nq_tot = 0
    for t in ORDER:
        ns_off[t] = ns_tot
        nq_off[t] = nq_tot
        js = slice(t * P, (t + 1) * P)
        for k in slow_ks[t]:
            ms_cols.append(MTf[k * P:(k + 1) * P, js])
        for k0 in fast_k0[t]:
            mf_cols.append(MTf[k0 * P:(k0 + 1) * P, js] * SC[t])
            mf_cols.append(MTf[(k0 + 1) * P:(k0 + 2) * P, js] * SC[t])
        ns_tot += len(slow_ks[t])
        nq_tot += len(fast_k0[t])

    ms_packed = (np.concatenate(ms_cols, axis=1) if ms_cols
                 else np.zeros((P, P), np.float32))
    mf_packed = (np.concatenate(mf_cols, axis=1) if mf_cols
                 else np.zeros((P, 2 * P), np.float32))
    ms_arr = ms_packed.astype(ml_dtypes.bfloat16)
    mf_arr = np.clip(mf_packed, -240, 240).astype(ml_dtypes.float8_e4m3)

    xb_full = xP.astype(ml_dtypes.bfloat16)
    x4_full = np.clip(xP * X4SC, -240, 240).astype(ml_dtypes.float8_e4m3)

    nc = _build_device_kernel(slow_ks, fast_k0, ns_off, nq_off, SC,
                              ns_tot, nq_tot)
    in_maps = [
        {
            "xb": np.ascontiguousarray(xb_full[c * BS:(c + 1) * BS].T),
            "x4": np.ascontiguousarray(x4_full[c * BS:(c + 1) * BS].T),
            "ms": ms_arr,
            "mf": mf_arr,
        }
        for c in range(NCORES)
    ]
    trace = bool(int(os.environ.get("KERNEL_TRACE", "0")))
    res = run_bass_kernel_spmd(
        nc, in_maps, core_ids=list(range(NCORES)), trace=trace
    )
    if trace and res.exec_time_ns is not None:
        print(f"HW exec time: {res.exec_time_ns} ns")
        if res.instructions_and_trace is not None:
            print(f"trace: {res.instructions_and_trace[1]}")

    outP = np.concatenate(
        [res.results[c]["outT"].astype(np.float32).T for c in range(NCORES)],
        axis=0,
    )
    return np.ascontiguousarray(
        outP.reshape(B, SIZE, SIZE).transpose(0, 2, 1).reshape(B, N)
    )
